# revision 28
# baseline (speedup 1.0000x reference)
"""Trainium2 Bass kernel for nn_ChiSquareLoss (histogram binning + chi-square).

Strategy (pure data parallel across 8 NeuronCores, 4 images/core):
  - Each core receives 24 "planes" of 512x512 fp32 pixels in [0,1):
    4 images x 3 channels x 2 input tensors, laid out as [24, 128, 2048].
  - Per plane, compute a 256-bin histogram via a factored one-hot:
      idx = floor(255*x) in [0,254] via the 2^23 RNE trick (exact; rare
      integer-z RNE ties corrected on host), hi = floor(idx/16) (exact,
      tie-free bias), lo = idx - 16*hi.
      one-hot masks (bf16): HIM[j] = (hi == j), LOM[i] = (lo == i), j,i in
      0..15, generated on DVE (is_equal, 4x mode) + ScalarE (Abs/Relu pair).
      hist2d[j,i] = sum_pixels HIM[j]*LOM[i]  -> TensorE outer-product
      matmuls, 8 pixel-columns packed per [128,128] bf16 matmul, accumulated
      in PSUM; the 8 stride-8 diagonal blocks of the PSUM tile hold hist2d.
  - Raw [128,128] accumulators are DMA'd out; host sums diagonal blocks,
    assembles [32, 768] histograms for both inputs and finishes the (tiny)
    chi-square + mean reduction exactly.
"""

import sys

if "/opt/trn_rl_repo" not in sys.path:
    sys.path.insert(0, "/opt/trn_rl_repo")

from contextlib import ExitStack

import numpy as np

import concourse.bacc as bacc
import concourse.bass as bass
import concourse.tile as tile
from concourse import mybir
from concourse.bass_utils import run_bass_kernel_spmd

ALU = mybir.AluOpType
F32 = mybir.dt.float32
BF16 = mybir.dt.bfloat16

B, C, H, W = 32, 3, 512, 512
NCORES = 8
IMGS = B // NCORES            # images per core
PLANES = IMGS * C * 2         # 24 planes per core (hist1 planes then hist2 planes)
P = 128                       # SBUF partitions
FREE = (H * W) // P           # 2048 pixel columns per plane
FCH = 1024                    # free-dim chunk size
NCH = FREE // FCH
PACK = 8                      # pixel columns packed per matmul
NBINS = 256
BIAS = 1e-10

# Mask-generation scheme: the matmul columns need not be one-hots -- any
# 16 linearly independent, exactly-representable functions per side work,
# with the host inverting the mixed basis to recover counts exactly.
#   hi side: thermometer columns [idx >= 16j] (DVE is_ge straight from idx,
#            no hi tensor needed); column j=0 is all-ones, pre-memset once.
#   lo side: |lo - i| distance columns for i < nabs (ScalarE, one Abs act
#            per column), one-hots [lo == i] for nabs <= i < 15 (DVE), and
#            a constant-ones column at 15, pre-memset once.
# nabs alternates by plane parity to balance DVE vs ScalarE load.
NABS_CYCLE = (10, 9)          # abs-column count by plane parity


def nabs_for(pl: int) -> int:
    # First/last planes run DVE-heavy: ScalarE starts late (waits on the
    # first lo) and finishes late (abs tails), while DVE idles there.
    if pl in (0, 1, 22, 23):
        return 6
    return NABS_CYCLE[pl % 2]
ONES_LO = 15                  # lo column pre-filled with constant ones
ONES_HI = 0                   # hi thermometer column j=0 (all ones)

_cache = {}


def build_kernel(planes=PLANES, free=FREE, fch=FCH):
    nc = bacc.Bacc()
    x_in = nc.declare_dram_parameter("x", [planes, P, free], F32, isOutput=False)
    h_out = nc.declare_dram_parameter("h", [planes, P, P], F32, isOutput=True)

    nch = free // fch
    npack = fch // PACK

    with ExitStack() as ctx:
        tc = ctx.enter_context(tile.TileContext(nc))
        const_pool = ctx.enter_context(tc.tile_pool(name="const", bufs=1))
        pix_pool = ctx.enter_context(tc.tile_pool(name="pix", bufs=2))
        tmp_pool = ctx.enter_context(tc.tile_pool(name="tmp", bufs=2))
        mask_pool = ctx.enter_context(tc.tile_pool(name="mask", bufs=2))
        psum_pool = ctx.enter_context(tc.tile_pool(name="ps", bufs=8, space="PSUM"))
        out_pool = ctx.enter_context(tc.tile_pool(name="hout", bufs=8))

        neg_bias = {}
        for j in range(max(NABS_CYCLE)):
            t = const_pool.tile([P, 1], F32, tag=f"nb{j}")
            nc.vector.memset(t, float(-j))
            neg_bias[j] = t
        # Pre-fill the constant-ones columns (ONES_LO / ONES_HI) in both
        # rotating lom/him buffers once; no per-chunk op rewrites them.
        for _ in range(2):
            lom_pre = mask_pool.tile([P, 16, npack, PACK], BF16, tag="lom")
            nc.vector.memset(lom_pre[:, ONES_LO], 1.0)
            him_pre = mask_pool.tile([P, npack, 16, PACK], BF16, tag="him")
            nc.vector.memset(him_pre[:, :, ONES_HI, :], 1.0)

        def emit_copy_out(pl, ps):
            # PSUM -> SBUF copy alternates engines to split the load; the
            # final plane's output rides the otherwise-idle sync queue to
            # shrink the kernel tail.
            hist_sb = out_pool.tile([P, P], F32, tag="hist")
            if pl % 2 == 0:
                nc.vector.tensor_copy(hist_sb, ps)
            else:
                nc.scalar.activation(hist_sb, ps, mybir.ActivationFunctionType.Copy)
            nc.sync.dma_start(out=h_out[pl], in_=hist_sb)

        prev = None
        for pl in range(planes):
            ps_bank = psum_pool.tile([P, 512], F32, tag="ps")
            ps = ps_bank[:, 0:P]

            # Per-plane prep at full width (FD=free): halves the per-op
            # fixed overhead (SBUF access bubble) on both DVE and ScalarE
            # relative to per-chunk prep. Plane 0 preps in halves to
            # shorten the initial cross-engine pipeline fill; the last
            # plane bins in quarter chunks so TensorE drains sooner.
            x_t = pix_pool.tile([P, free], F32, tag="x")
            if pl == 0:
                nc.sync.dma_start(out=x_t[:, 0:fch], in_=x_in[pl, :, 0:fch])
                nc.sync.dma_start(out=x_t[:, fch:free], in_=x_in[pl, :, fch:free])
            else:
                nc.sync.dma_start(out=x_t, in_=x_in[pl, :, :])

            zh = tmp_pool.tile([P, free], F32, tag="zh")
            idx = tmp_pool.tile([P, free], BF16, tag="idx")
            hi16 = tmp_pool.tile([P, free], BF16, tag="hi16")
            lo_full = tmp_pool.tile([P, free], BF16, tag="lo")

            psegs = [(0, free)] if pl > 0 else [(0, fch), (fch, free - fch)]
            for so, sn in psegs:
                sl = slice(so, so + sn)
                # Exact floor via the 2^23 round-to-nearest trick:
                #   zh  = z - 0.5 (exact; z = RNE(255*x))
                #   idx = RNE(zh + 2^23+8) - (2^23+8) = floor(z); RNE tie
                #         only when z is exactly an integer k: k odd -> k-1
                #         (host-corrected).
                #   u   = idx/16 - 0.46875 (exact, on the 2^-5 grid, never
                #         a tie) -> hi = RNE(u + 2^23+8) - (2^23+8)
                #         = floor(idx/16) exactly.
                #   lo  = idx - 16*hi (exact).
                nc.vector.tensor_scalar(
                    zh[:, sl], x_t[:, sl], 255.0, -0.5, ALU.mult, ALU.add
                )
                nc.vector.tensor_scalar(
                    idx[:, sl], zh[:, sl], 8388616.0, -8388616.0, ALU.add, ALU.add
                )
                # hi16 = 16*floor(idx/16) via RNE on the fp32 lsb-16 grid:
                # idx - 7.5 + 1.5*2^27 rounds to the nearest multiple of 16
                # (the whole range stays in [2^27, 2^28) where lsb = 16),
                # tie-free for integer idx. Two DVE passes, reusing zh.
                nc.vector.tensor_scalar(
                    zh[:, sl], idx[:, sl], 7.5, 201326592.0,
                    ALU.subtract, ALU.add,
                )
                nc.vector.tensor_scalar(
                    hi16[:, sl], zh[:, sl], -201326592.0, None, ALU.add
                )
                nc.vector.tensor_tensor(
                    lo_full[:, sl], idx[:, sl], hi16[:, sl], ALU.subtract
                )

            if pl == planes - 1:
                csz = fch // 2
                chunks = [(k * csz, csz) for k in range(free // csz)]
            else:
                chunks = [(k * fch, fch) for k in range(nch)]
            nabs = nabs_for(pl)
            for ci, (co, cn) in enumerate(chunks):
                ix = idx[:, co:co + cn]
                lo = lo_full[:, co:co + cn]
                npc = cn // PACK

                # him: pack-interleaved layout [P, npack, 16 bins, PACK cols]
                # so each matmul weight slab him[:, s] is contiguous (fast
                # weight load). lom: bin-major [P, 16, npack, PACK] -- the
                # moving operand has no contiguity requirement, and this
                # makes each mask write (and the DMA-refreshed constant-ones
                # column) a contiguous [P, cn] store.
                # Mask work is split between DVE (is_equal one-hots, 4x
                # mode), ScalarE (|lo - i| distance columns, one Abs act
                # each), and the sync DMA queue (the ones column); mask
                # values are exact small integers in bf16, and the host
                # inverts the mixed one-hot/ones/distance system exactly.
                him_t = mask_pool.tile([P, npack, 16, PACK], BF16, tag="him")
                lom_t = mask_pool.tile([P, 16, npack, PACK], BF16, tag="lom")
                him = him_t[:, :npc]
                ix_r = ix.rearrange("p (s t) -> p s t", t=PACK)
                lo_r = lo.rearrange("p (s t) -> p s t", t=PACK)

                def abs_mask(dst, src_r, j):
                    nc.scalar.activation(
                        dst, src_r, mybir.ActivationFunctionType.Abs,
                        bias=neg_bias[j][:, 0:1],
                    )

                for j in range(1, 16):
                    nc.vector.tensor_scalar(
                        him[:, :, j, :], ix_r, float(16 * j), None, ALU.is_ge
                    )
                for i in range(nabs):
                    abs_mask(lom_t[:, i, :npc, :], lo_r, i)
                for i in range(nabs, 15):
                    nc.vector.tensor_scalar(
                        lom_t[:, i, :npc, :], lo_r, float(i), None, ALU.is_equal
                    )

                # Deferred copy-out of the previous plane's accumulator:
                # emitted here (mid-plane) so the issuing engine's FIFO
                # never stalls waiting on TensorE to drain.
                if ci == 0 and prev is not None:
                    emit_copy_out(*prev)
                    prev = None

                for s in range(npc):
                    lhsT = him[:, s].rearrange("p j t -> p (j t)")
                    rhs = lom_t[:, :, s, :]
                    nc.tensor.matmul(
                        ps,
                        lhsT,
                        rhs,
                        start=(ci == 0 and s == 0),
                        stop=(ci == len(chunks) - 1 and s == npc - 1),
                    )

            prev = (pl, ps)

        emit_copy_out(*prev)

    nc.finalize()
    return nc


def _get_nc():
    if "nc" not in _cache:
        _cache["nc"] = build_kernel()
    return _cache["nc"]


def shard_inputs(hist1: np.ndarray, hist2: np.ndarray):
    """Build per-core input maps: core i gets images [4i, 4i+4) of both tensors."""
    in_maps = []
    for i in range(NCORES):
        sl1 = hist1[i * IMGS:(i + 1) * IMGS]  # [4, 3, 512, 512]
        sl2 = hist2[i * IMGS:(i + 1) * IMGS]
        x = np.concatenate(
            [
                np.ascontiguousarray(sl1).reshape(IMGS * C, P, FREE),
                np.ascontiguousarray(sl2).reshape(IMGS * C, P, FREE),
            ],
            axis=0,
        )  # [24, 128, 2048]
        in_maps.append({"x": np.ascontiguousarray(x, dtype=np.float32)})
    return in_maps


def _lo_basis_inv(nabs: int) -> np.ndarray:
    """Inverse of the lo-column basis: distance rows |l - i| for i < nabs,
    one-hot rows for nabs <= i < 15, constant-ones row at 15."""
    V = np.zeros((16, 16))
    for i in range(nabs):
        V[i, :] = np.abs(np.arange(16) - i)
    for i in range(nabs, 15):
        V[i, i] = 1.0
    V[15, :] = 1.0
    return np.linalg.inv(V)


def _hi_basis_inv() -> np.ndarray:
    # thermometer basis: row j = [h >= j]
    V = np.fromfunction(lambda j, h: (h >= j) * 1.0, (16, 16))
    return np.linalg.inv(V)


_VLINV = {n: _lo_basis_inv(n) for n in {nabs_for(p) for p in range(PLANES)}}
_VHINV = _hi_basis_inv()


def hist2d_from_raw(raw: np.ndarray, plane: int) -> np.ndarray:
    """raw: [128, 128] PSUM accumulator of plane `plane` -> [256] histogram.

    PSUM row m = 8*hi_col + t, col n = 8*lo_col + t'; useful entries on the
    t == t' diagonals: A[m, i, t] = Vh[m, h] c[h, l, t] Vl[i, l] with Vh/Vl
    the mixed one-hot/ones/distance bases. Counts are exact integers; the
    solve error is ~1e-8, so rounding recovers them exactly.
    """
    vlinv = _VLINV[nabs_for(plane)]
    r = raw.reshape(16, PACK, 16, PACK).astype(np.float64)
    A = np.einsum("mtit->mit", r)                  # [16 hi-cols, 16 lo-cols, PACK]
    C = np.einsum("hm,li,mit->hlt", _VHINV, vlinv, A)
    return np.rint(C).sum(axis=-1).reshape(NBINS)


def fixup_hist(hist: np.ndarray, plane_x: np.ndarray) -> None:
    """Correct the RNE tie cases in-place so counts match exact floor binning.

    Device semantics: for z = fl(255*x) exactly an odd integer k, the RNE tie
    binned the pixel at k-1 instead of k. All other pixels are binned exactly.
    """
    z = plane_x.astype(np.float32) * np.float32(255.0)
    zf = z[z == np.floor(z)]
    if zf.size == 0:
        return
    k = zf.astype(np.int64)
    odd = k[k % 2 == 1]
    for kk, cnt in zip(*np.unique(odd, return_counts=True)):
        hist[kk - 1] -= cnt
        hist[kk] += cnt


def finish_on_host(per_core_hists: list) -> np.ndarray:
    """per_core_hists: NCORES arrays [24, 256] -> scalar chi-square loss."""
    h = np.stack(per_core_hists)  # [8, 24, 256]
    h = h.reshape(NCORES, 2, IMGS, C, NBINS)
    counts1 = h[:, 0].reshape(B, C * NBINS)  # [32, 768]
    counts2 = h[:, 1].reshape(B, C * NBINS)
    n = float(C * H * W)
    h1 = counts1 / n
    h2 = counts2 / n
    chi = np.sum((h1 - h2) ** 2 / (h1 + h2 + BIAS), axis=1)
    return np.array(np.mean(chi), dtype=np.float32)


def kernel(hist1: np.ndarray, hist2: np.ndarray) -> np.ndarray:
    hist1 = np.asarray(hist1, dtype=np.float32)
    hist2 = np.asarray(hist2, dtype=np.float32)
    nc = _get_nc()
    in_maps = shard_inputs(hist1, hist2)
    res = run_bass_kernel_spmd(nc, in_maps, list(range(NCORES)))
    per_core = []
    for i in range(NCORES):
        raw = res.results[i]["h"]
        hists = np.stack(
            [hist2d_from_raw(raw[pl], pl) for pl in range(PLANES)]
        )  # [24, 256]
        for pl in range(PLANES):
            fixup_hist(hists[pl], in_maps[i]["x"][pl])
        per_core.append(hists)
    return finish_on_host(per_core)


if __name__ == "__main__":
    rng = np.random.default_rng(0)
    h1 = rng.random((B, C, H, W), dtype=np.float32)
    h2 = rng.random((B, C, H, W), dtype=np.float32)
    out = kernel(h1, h2)
    print("kernel output:", out)



# revision 29
# speedup vs baseline: 1.1968x; 1.1968x over previous
"""Trainium2 Bass kernel for nn_ChiSquareLoss (histogram binning + chi-square).

Strategy (pure data parallel across 8 NeuronCores, 4 images/core):
  - Each core receives 24 "planes" of 512x512 fp32 pixels in [0,1):
    4 images x 3 channels x 2 input tensors, laid out as [24, 128, 2048].
  - Per plane, compute a 256-bin histogram via a factored one-hot:
      idx = floor(255*x) in [0,254] via the 2^23 RNE trick (exact; rare
      integer-z RNE ties corrected on host), hi = floor(idx/16) (exact,
      tie-free bias), lo = idx - 16*hi.
      one-hot masks (bf16): HIM[j] = (hi == j), LOM[i] = (lo == i), j,i in
      0..15, generated on DVE (is_equal, 4x mode) + ScalarE (Abs/Relu pair).
      hist2d[j,i] = sum_pixels HIM[j]*LOM[i]  -> TensorE outer-product
      matmuls, 8 pixel-columns packed per [128,128] bf16 matmul, accumulated
      in PSUM; the 8 stride-8 diagonal blocks of the PSUM tile hold hist2d.
  - Raw [128,128] accumulators are DMA'd out; host sums diagonal blocks,
    assembles [32, 768] histograms for both inputs and finishes the (tiny)
    chi-square + mean reduction exactly.
"""

import sys

if "/opt/trn_rl_repo" not in sys.path:
    sys.path.insert(0, "/opt/trn_rl_repo")

from contextlib import ExitStack

import numpy as np

import concourse.bacc as bacc
import concourse.bass as bass
import concourse.tile as tile
from concourse import mybir
from concourse.bass_utils import run_bass_kernel_spmd

ALU = mybir.AluOpType
F32 = mybir.dt.float32
BF16 = mybir.dt.bfloat16

B, C, H, W = 32, 3, 512, 512
NCORES = 8
IMGS = B // NCORES            # images per core
PLANES = IMGS * C * 2         # 24 planes per core (hist1 planes then hist2 planes)
P = 128                       # SBUF partitions
FREE = (H * W) // P           # 2048 pixel columns per plane
FCH = 1024                    # free-dim chunk size
NCH = FREE // FCH
PACK = 8                      # pixel columns packed per matmul
NBINS = 256
BIAS = 1e-10

# Mask-generation scheme: the matmul columns need not be one-hots -- any
# 16 linearly independent, exactly-representable functions per side work,
# with the host inverting the mixed basis to recover counts exactly.
#   hi side: thermometer columns [idx >= 16j] (DVE is_ge straight from idx,
#            no hi tensor needed); column j=0 is all-ones, pre-memset once.
#   lo side: |lo - i| distance columns for i < nabs (ScalarE, one Abs act
#            per column), one-hots [lo == i] for nabs <= i < 15 (DVE), and
#            a constant-ones column at 15, pre-memset once.
# nabs alternates by plane parity to balance DVE vs ScalarE load.
NABS_CYCLE = (10, 9)          # abs-column count by plane parity


def nabs_for(pl: int) -> int:
    # First/last planes run DVE-heavy: ScalarE starts late (waits on the
    # first lo) and finishes late (abs tails), while DVE idles there.
    return NABS_CYCLE[pl % 2]
ONES_LO = 15                  # lo column pre-filled with constant ones
ONES_HI = 0                   # hi thermometer column j=0 (all ones)

_cache = {}


def build_kernel(planes=PLANES, free=FREE, fch=FCH):
    nc = bacc.Bacc()
    x_in = nc.declare_dram_parameter("x", [planes, P, free], F32, isOutput=False)
    h_out = nc.declare_dram_parameter("h", [planes, P, P], F32, isOutput=True)

    nch = free // fch
    npack = fch // PACK

    with ExitStack() as ctx:
        tc = ctx.enter_context(tile.TileContext(nc))
        const_pool = ctx.enter_context(tc.tile_pool(name="const", bufs=1))
        pix_pool = ctx.enter_context(tc.tile_pool(name="pix", bufs=2))
        tmp_pool = ctx.enter_context(tc.tile_pool(name="tmp", bufs=2))
        mask_pool = ctx.enter_context(tc.tile_pool(name="mask", bufs=2))
        psum_pool = ctx.enter_context(tc.tile_pool(name="ps", bufs=8, space="PSUM"))
        out_pool = ctx.enter_context(tc.tile_pool(name="hout", bufs=8))

        neg_bias = {}
        for j in range(max(NABS_CYCLE)):
            t = const_pool.tile([P, 1], F32, tag=f"nb{j}")
            nc.vector.memset(t, float(-j))
            neg_bias[j] = t
        # Pre-fill the constant-ones columns (ONES_LO / ONES_HI) in both
        # rotating lom/him buffers once; no per-chunk op rewrites them.
        for _ in range(2):
            lom_pre = mask_pool.tile([P, 16, npack, PACK], BF16, tag="lom")
            nc.vector.memset(lom_pre[:, ONES_LO], 1.0)
            him_pre = mask_pool.tile([P, npack, 16, PACK], BF16, tag="him")
            nc.vector.memset(him_pre[:, :, ONES_HI, :], 1.0)

        def emit_copy_out(pl, ps):
            # PSUM -> SBUF copy alternates engines to split the load; the
            # final plane's output rides the otherwise-idle sync queue to
            # shrink the kernel tail.
            hist_sb = out_pool.tile([P, P], F32, tag="hist")
            if pl % 2 == 0:
                nc.vector.tensor_copy(hist_sb, ps)
            else:
                nc.scalar.activation(hist_sb, ps, mybir.ActivationFunctionType.Copy)
            nc.sync.dma_start(out=h_out[pl], in_=hist_sb)

        prev = None
        for pl in range(planes):
            ps_bank = psum_pool.tile([P, 512], F32, tag="ps")
            ps = ps_bank[:, 0:P]

            # Per-plane prep at full width (FD=free): halves the per-op
            # fixed overhead (SBUF access bubble) on both DVE and ScalarE
            # relative to per-chunk prep. Plane 0 preps in halves to
            # shorten the initial cross-engine pipeline fill; the last
            # plane bins in quarter chunks so TensorE drains sooner.
            x_t = pix_pool.tile([P, free], F32, tag="x")
            if pl == 0:
                nc.sync.dma_start(out=x_t[:, 0:fch], in_=x_in[pl, :, 0:fch])
                nc.sync.dma_start(out=x_t[:, fch:free], in_=x_in[pl, :, fch:free])
            else:
                nc.sync.dma_start(out=x_t, in_=x_in[pl, :, :])

            zh = tmp_pool.tile([P, free], F32, tag="zh")
            idx = tmp_pool.tile([P, free], BF16, tag="idx")
            hi16 = tmp_pool.tile([P, free], BF16, tag="hi16")
            lo_full = tmp_pool.tile([P, free], BF16, tag="lo")

            psegs = [(0, free)] if pl > 0 else [(0, fch), (fch, free - fch)]
            for so, sn in psegs:
                sl = slice(so, so + sn)
                # Exact floor via the 2^23 round-to-nearest trick:
                #   zh  = z - 0.5 (exact; z = RNE(255*x))
                #   idx = RNE(zh + 2^23+8) - (2^23+8) = floor(z); RNE tie
                #         only when z is exactly an integer k: k odd -> k-1
                #         (host-corrected).
                #   u   = idx/16 - 0.46875 (exact, on the 2^-5 grid, never
                #         a tie) -> hi = RNE(u + 2^23+8) - (2^23+8)
                #         = floor(idx/16) exactly.
                #   lo  = idx - 16*hi (exact).
                nc.vector.tensor_scalar(
                    zh[:, sl], x_t[:, sl], 255.0, -0.5, ALU.mult, ALU.add
                )
                nc.vector.tensor_scalar(
                    idx[:, sl], zh[:, sl], 8388616.0, -8388616.0, ALU.add, ALU.add
                )
                # hi16 = 16*floor(idx/16) via RNE on the fp32 lsb-16 grid:
                # idx - 7.5 + 1.5*2^27 rounds to the nearest multiple of 16
                # (the whole range stays in [2^27, 2^28) where lsb = 16),
                # tie-free for integer idx. Two DVE passes, reusing zh.
                nc.vector.tensor_scalar(
                    zh[:, sl], idx[:, sl], 7.5, 201326592.0,
                    ALU.subtract, ALU.add,
                )
                nc.vector.tensor_scalar(
                    hi16[:, sl], zh[:, sl], -201326592.0, None, ALU.add
                )
                nc.vector.tensor_tensor(
                    lo_full[:, sl], idx[:, sl], hi16[:, sl], ALU.subtract
                )

            if pl == planes - 1:
                csz = fch // 2
                chunks = [(k * csz, csz) for k in range(free // csz)]
            else:
                chunks = [(k * fch, fch) for k in range(nch)]
            nabs = nabs_for(pl)
            for ci, (co, cn) in enumerate(chunks):
                ix = idx[:, co:co + cn]
                lo = lo_full[:, co:co + cn]
                npc = cn // PACK

                # him: pack-interleaved layout [P, npack, 16 bins, PACK cols]
                # so each matmul weight slab him[:, s] is contiguous (fast
                # weight load). lom: bin-major [P, 16, npack, PACK] -- the
                # moving operand has no contiguity requirement, and this
                # makes each mask write (and the DMA-refreshed constant-ones
                # column) a contiguous [P, cn] store.
                # Mask work is split between DVE (is_equal one-hots, 4x
                # mode), ScalarE (|lo - i| distance columns, one Abs act
                # each), and the sync DMA queue (the ones column); mask
                # values are exact small integers in bf16, and the host
                # inverts the mixed one-hot/ones/distance system exactly.
                him_t = mask_pool.tile([P, npack, 16, PACK], BF16, tag="him")
                lom_t = mask_pool.tile([P, 16, npack, PACK], BF16, tag="lom")
                him = him_t[:, :npc]
                ix_r = ix.rearrange("p (s t) -> p s t", t=PACK)
                lo_r = lo.rearrange("p (s t) -> p s t", t=PACK)

                def abs_mask(dst, src_r, j):
                    nc.scalar.activation(
                        dst, src_r, mybir.ActivationFunctionType.Abs,
                        bias=neg_bias[j][:, 0:1],
                    )

                for j in range(1, 16):
                    nc.vector.tensor_scalar(
                        him[:, :, j, :], ix_r, float(16 * j), None, ALU.is_ge
                    )
                for i in range(nabs):
                    abs_mask(lom_t[:, i, :npc, :], lo_r, i)
                for i in range(nabs, 15):
                    nc.vector.tensor_scalar(
                        lom_t[:, i, :npc, :], lo_r, float(i), None, ALU.is_equal
                    )

                # Deferred copy-out of the previous plane's accumulator:
                # emitted here (mid-plane) so the issuing engine's FIFO
                # never stalls waiting on TensorE to drain.
                if ci == 0 and prev is not None:
                    emit_copy_out(*prev)
                    prev = None

                for s in range(npc):
                    lhsT = him[:, s].rearrange("p j t -> p (j t)")
                    rhs = lom_t[:, :, s, :]
                    nc.tensor.matmul(
                        ps,
                        lhsT,
                        rhs,
                        start=(ci == 0 and s == 0),
                        stop=(ci == len(chunks) - 1 and s == npc - 1),
                    )

            prev = (pl, ps)

        emit_copy_out(*prev)

    nc.finalize()
    return nc


def _get_nc():
    if "nc" not in _cache:
        _cache["nc"] = build_kernel()
    return _cache["nc"]


def shard_inputs(hist1: np.ndarray, hist2: np.ndarray):
    """Build per-core input maps: core i gets images [4i, 4i+4) of both tensors."""
    in_maps = []
    for i in range(NCORES):
        sl1 = hist1[i * IMGS:(i + 1) * IMGS]  # [4, 3, 512, 512]
        sl2 = hist2[i * IMGS:(i + 1) * IMGS]
        x = np.concatenate(
            [
                np.ascontiguousarray(sl1).reshape(IMGS * C, P, FREE),
                np.ascontiguousarray(sl2).reshape(IMGS * C, P, FREE),
            ],
            axis=0,
        )  # [24, 128, 2048]
        in_maps.append({"x": np.ascontiguousarray(x, dtype=np.float32)})
    return in_maps


def _lo_basis_inv(nabs: int) -> np.ndarray:
    """Inverse of the lo-column basis: distance rows |l - i| for i < nabs,
    one-hot rows for nabs <= i < 15, constant-ones row at 15."""
    V = np.zeros((16, 16))
    for i in range(nabs):
        V[i, :] = np.abs(np.arange(16) - i)
    for i in range(nabs, 15):
        V[i, i] = 1.0
    V[15, :] = 1.0
    return np.linalg.inv(V)


def _hi_basis_inv() -> np.ndarray:
    # thermometer basis: row j = [h >= j]
    V = np.fromfunction(lambda j, h: (h >= j) * 1.0, (16, 16))
    return np.linalg.inv(V)


_VLINV = {n: _lo_basis_inv(n) for n in {nabs_for(p) for p in range(PLANES)}}
_VHINV = _hi_basis_inv()


def hist2d_from_raw(raw: np.ndarray, plane: int) -> np.ndarray:
    """raw: [128, 128] PSUM accumulator of plane `plane` -> [256] histogram.

    PSUM row m = 8*hi_col + t, col n = 8*lo_col + t'; useful entries on the
    t == t' diagonals: A[m, i, t] = Vh[m, h] c[h, l, t] Vl[i, l] with Vh/Vl
    the mixed one-hot/ones/distance bases. Counts are exact integers; the
    solve error is ~1e-8, so rounding recovers them exactly.
    """
    vlinv = _VLINV[nabs_for(plane)]
    r = raw.reshape(16, PACK, 16, PACK).astype(np.float64)
    A = np.einsum("mtit->mit", r)                  # [16 hi-cols, 16 lo-cols, PACK]
    C = np.einsum("hm,li,mit->hlt", _VHINV, vlinv, A)
    return np.rint(C).sum(axis=-1).reshape(NBINS)


def fixup_hist(hist: np.ndarray, plane_x: np.ndarray) -> None:
    """Correct the RNE tie cases in-place so counts match exact floor binning.

    Device semantics: for z = fl(255*x) exactly an odd integer k, the RNE tie
    binned the pixel at k-1 instead of k. All other pixels are binned exactly.
    """
    z = plane_x.astype(np.float32) * np.float32(255.0)
    zf = z[z == np.floor(z)]
    if zf.size == 0:
        return
    k = zf.astype(np.int64)
    odd = k[k % 2 == 1]
    for kk, cnt in zip(*np.unique(odd, return_counts=True)):
        hist[kk - 1] -= cnt
        hist[kk] += cnt


def finish_on_host(per_core_hists: list) -> np.ndarray:
    """per_core_hists: NCORES arrays [24, 256] -> scalar chi-square loss."""
    h = np.stack(per_core_hists)  # [8, 24, 256]
    h = h.reshape(NCORES, 2, IMGS, C, NBINS)
    counts1 = h[:, 0].reshape(B, C * NBINS)  # [32, 768]
    counts2 = h[:, 1].reshape(B, C * NBINS)
    n = float(C * H * W)
    h1 = counts1 / n
    h2 = counts2 / n
    chi = np.sum((h1 - h2) ** 2 / (h1 + h2 + BIAS), axis=1)
    return np.array(np.mean(chi), dtype=np.float32)


def kernel(hist1: np.ndarray, hist2: np.ndarray) -> np.ndarray:
    hist1 = np.asarray(hist1, dtype=np.float32)
    hist2 = np.asarray(hist2, dtype=np.float32)
    nc = _get_nc()
    in_maps = shard_inputs(hist1, hist2)
    res = run_bass_kernel_spmd(nc, in_maps, list(range(NCORES)))
    per_core = []
    for i in range(NCORES):
        raw = res.results[i]["h"]
        hists = np.stack(
            [hist2d_from_raw(raw[pl], pl) for pl in range(PLANES)]
        )  # [24, 256]
        for pl in range(PLANES):
            fixup_hist(hists[pl], in_maps[i]["x"][pl])
        per_core.append(hists)
    return finish_on_host(per_core)


if __name__ == "__main__":
    rng = np.random.default_rng(0)
    h1 = rng.random((B, C, H, W), dtype=np.float32)
    h2 = rng.random((B, C, H, W), dtype=np.float32)
    out = kernel(h1, h2)
    print("kernel output:", out)

